# revision 48
# baseline (speedup 1.0000x reference)
"""DocumentEncoder kernel for Trainium2 (8 NeuronCores, Bass/Tile).

Reference computation (B=256, L=512, D=1024, V=50000):
    emb    = emb_table[tokens + 1]            # [B, L, D] gather
    hidden = emb.mean(axis=1)                 # [B, D]
    ha     = einsum('bld,de->ble', emb, W_b)
    scores = einsum('ble,be->bl', ha, hidden)
    attn   = softmax(scores, axis=1)
    ct     = einsum('bl,bld->bd', attn, emb)  # [B, D]

Key algebraic rewrite: scores[b,l] = emb[b,l,:] . (W_b @ hidden[b]) -- the
[B,L,D]x[D,D] einsum collapses to a [B,D]x[D,D] one (~230x less compute),
leaving the embedding-row gather as the dominant cost.

Sharding: data-parallel over B (32 docs per core), table/W_b replicated.

The device-side gather uses the DMAGatherAnt SWDGE ucode, whose indices are
int16: each core therefore gets a host-built compact table holding only the
(<= 16384, so int16-safe) embedding rows its own docs reference, with token
ids remapped accordingly.

Per-core device program, groups of G=4 docs:
  1. dma_gather per 128-token block: 128 rows -> E [128p, 1, 1024d]
     (token l at partition l%128); 30 block-sized SBUF slots give the
     gather a deep prefetch window and fine-grained slot recycling
  2. s_j = sum_l emb_j on PE as a [1, 1024] row (fp32r, ones stationary,
     E moving)
  3. PE-transpose of the 4 rows into s^T columns of one [128, 32] tile;
     batched V = (W_b^T/L)^T s contraction for the 4 docs (stride-8
     stationary)
  4. per doc, streamed per 128-token block: PE one-hot broadcast of v_j
     across partitions (both halves up front); then for each block c: DVE
     fused multiply+accumulate for scores_c, ACT exp with fused
     per-partition Z accumulation, and the block's term of the
     unnormalized context sum on PE (exp stationary, E moving) -- so each
     E block is released as soon as its term is in. Finally a tiny PE
     ones-matmul for the total Z, DVE reciprocal, ACT scale by 1/Z, and
     one 4KB DMA out per doc.

All PE matmuls run in float32r (fp32 bits, reduced-precision PE mode, 1
cycle/row at N>=256 vs 4 for full fp32); the end-to-end error vs the fp32
reference is ~1e-3 of the output scale.
"""

from contextlib import ExitStack

import numpy as np

import concourse.tile as tile
from concourse import bacc, mybir
from concourse.bass_utils import run_bass_kernel_spmd
from concourse.masks import make_identity

B, L, D, V = 256, 512, 1024, 50000
N_CORES = 8
DOCS = B // N_CORES          # 32 docs per core
G = 4                        # docs per group (col-tiling width)
NG = DOCS // G               # 8 groups
CB = L // 128                # 4 column blocks of 128 tokens per doc

FP32 = mybir.dt.float32
FP32R = mybir.dt.float32r
INT16 = mybir.dt.int16
U_MAX = DOCS * L             # compact per-core table rows (16384 < 2**15)
IW = L // 16                 # int16 index columns per doc (32)


def build_program():
    nc = bacc.Bacc(
        "TRN2",
        target_bir_lowering=False,
        debug=False,
        num_devices=N_CORES,
    )

    table = nc.dram_tensor("table", [U_MAX, D], FP32R, kind="ExternalInput").ap()
    wt = nc.dram_tensor("wt", [D, D], FP32R, kind="ExternalInput").ap()
    idx = nc.dram_tensor("idx", [128, DOCS * (L // 16)], INT16, kind="ExternalInput").ap()
    onehot = nc.dram_tensor("onehot", [G, G * 128], FP32R, kind="ExternalInput").ap()
    out = nc.dram_tensor("out", [DOCS, D], FP32, kind="ExternalOutput").ap()

    with tile.TileContext(nc) as tc, ExitStack() as ctx:
        const = ctx.enter_context(tc.tile_pool(name="const", bufs=1))
        wtp = ctx.enter_context(tc.tile_pool(name="wtp", bufs=1))
        ep = ctx.enter_context(tc.tile_pool(name="ep", bufs=30))
        sb2 = ctx.enter_context(tc.tile_pool(name="sb2", bufs=2))
        stp = ctx.enter_context(tc.tile_pool(name="stp", bufs=4))
        spv = ctx.enter_context(tc.tile_pool(name="spv", bufs=2, space="PSUM"))
        tvp = ctx.enter_context(tc.tile_pool(name="tvp", bufs=3, space="PSUM"))
        uzp = ctx.enter_context(tc.tile_pool(name="uzp", bufs=3, space="PSUM"))

        # ---- static tiles ----
        idx_sb = const.tile([128, DOCS * IW], INT16, tag="idx")
        nc.sync.dma_start(out=idx_sb[:], in_=idx[:])

        wt_sb = []
        for k in range(8):
            t = wtp.tile([128, D], FP32R, tag=f"wt{k}", name=f"wtt{k}")
            nc.sync.dma_start(out=t[:], in_=wt[k * 128 : (k + 1) * 128, :])
            wt_sb.append(t)

        oh_sb = const.tile([G, G * 128], FP32R, tag="oh")
        nc.sync.dma_start(out=oh_sb[:], in_=onehot[:])

        ones1 = const.tile([128, 1], FP32, tag="ones1")
        nc.vector.memset(ones1[:], 1.0)
        ones1r = const.tile([128, 1], FP32R, tag="ones1r")
        nc.scalar.copy(out=ones1r[:], in_=ones1[:])
        ident = const.tile([128, 128], FP32, tag="ident")
        make_identity(nc, ident[:])

        # small leading groups start the score pipeline (DVE) early; the
        # DVE backlog that would otherwise drain after the last gather
        # shrinks by the same amount
        schedule = [(0, 1), (1, 1), (2, 2)]
        schedule += [(4 + g * G, G) for g in range(NG - 1)]
        for base, gs in schedule:
            # ---- gather the group's embedding rows (per-128-token blocks) ----
            e_tiles = []
            for j in range(gs):
                b = base + j
                quarters = []
                for qq in range(CB):
                    eq = ep.tile([128, 1, D], FP32R, tag="e", name="eq")
                    nc.gpsimd.dma_gather(
                        out_ap=eq[:],
                        in_ap=table[:],
                        idxs_ap=idx_sb[
                            :, b * IW + qq * (IW // CB) : b * IW + (qq + 1) * (IW // CB)
                        ],
                        num_idxs=L // CB,
                        num_idxs_reg=L // CB,
                        elem_size=D,
                    )
                    quarters.append(eq)
                e_tiles.append(quarters)

            # ---- s_j = sum_l emb_j as a [1, 1024] row (per doc, M=1) ----
            s_rows = []
            for j in range(gs):
                srow = sb2.tile([1, D], FP32, tag="srow", bufs=4)
                for n in range(2):
                    sp = spv.tile([1, 512], FP32, tag="spv")
                    for c in range(CB):
                        nc.tensor.matmul(
                            out=sp[0:1, :],
                            lhsT=ones1r[:],
                            rhs=e_tiles[j][c][:, 0, n * 512 : (n + 1) * 512],
                            start=(c == 0),
                            stop=(c == CB - 1),
                        )
                    nc.scalar.copy(
                        out=srow[0:1, n * 512 : (n + 1) * 512], in_=sp[0:1, :]
                    )
                s_rows.append(srow)

            # ---- transpose the 4 rows into s^T columns: T[p, j*8+k] ----
            t_ps = tvp.tile([128, G * 8], FP32, tag="tv")
            for j in range(gs):
                for k in range(8):
                    col = j * 8 + k
                    nc.tensor.transpose(
                        out=t_ps[:, col : col + 1],
                        in_=s_rows[j][0:1, k * 128 : (k + 1) * 128],
                        identity=ident[0:1, 0:1],
                    )
            st_sb = sb2.tile([128, G * 8], FP32R, tag="st_sb")
            nc.scalar.copy(out=st_sb[:, 0 : gs * 8], in_=t_ps[:, 0 : gs * 8])

            # ---- V[j, d] = sum_e s_j[e] * (W^T/L)[e, d]  (group batched) ----
            v_sb = sb2.tile([G, D], FP32R, tag="v_sb")
            for n in range(2):
                vh = spv.tile([G, 512], FP32, tag="spv", name="vh")
                for k in range(8):
                    nc.tensor.matmul(
                        out=vh[0:gs, :],
                        lhsT=st_sb[:, k : k + 8 * (gs - 1) + 1 : 8],
                        rhs=wt_sb[k][:, n * 512 : (n + 1) * 512],
                        start=(k == 0),
                        stop=(k == 7),
                    )
                nc.scalar.copy(
                    out=v_sb[0:gs, n * 512 : (n + 1) * 512], in_=vh[0:gs, :]
                )

            # ---- per-doc epilogue, streamed per 128-token block ----
            # scores, exp, and the context-sum term of block c complete as a
            # unit, so the E quarter-tile for block c is released ~4x sooner
            # than with whole-doc phases; only the final 1/Z scale waits for
            # the full softmax denominator.
            for j in range(gs):
                b = base + j
                et = e_tiles[j]
                vbs = []
                for n in range(2):
                    vb = tvp.tile([128, 512], FP32, tag="tv", name="vb")
                    nc.tensor.matmul(
                        out=vb[:],
                        lhsT=oh_sb[0:gs, j * 128 : (j + 1) * 128],
                        rhs=v_sb[0:gs, n * 512 : (n + 1) * 512],
                        start=True,
                        stop=True,
                    )
                    vbs.append(vb)
                p_sb = sb2.tile([128, CB], FP32R, tag="p_sb", bufs=4)
                zp = sb2.tile([128, CB], FP32, tag="zp", bufs=4)
                u_h = [
                    uzp.tile([1, 512], FP32, tag="uz", name=f"u{h}") for h in range(2)
                ]
                for c in range(CB):
                    sc_c = sb2.tile([128, 2], FP32, tag="sc_c", bufs=8)
                    for n in range(2):
                        scr = sb2.tile([128, 512], FP32, tag="scr")
                        nc.vector.scalar_tensor_tensor(
                            out=scr[:],
                            in0=et[c][:, 0, n * 512 : (n + 1) * 512].bitcast(FP32),
                            scalar=1.0,
                            in1=vbs[n][:],
                            op0=mybir.AluOpType.mult,
                            op1=mybir.AluOpType.mult,
                            accum_out=sc_c[:, n : n + 1],
                        )
                    scores_c = sb2.tile([128, 1], FP32, tag="scores_c", bufs=8)
                    nc.vector.tensor_tensor(
                        out=scores_c[:],
                        in0=sc_c[:, 0:1],
                        in1=sc_c[:, 1:2],
                        op=mybir.AluOpType.add,
                    )
                    nc.scalar.activation(
                        out=p_sb[:, c : c + 1],
                        in_=scores_c[:],
                        func=mybir.ActivationFunctionType.Exp,
                        accum_out=zp[:, c : c + 1],
                    )
                    for h in range(2):
                        nc.tensor.matmul(
                            out=u_h[h][0:1, :],
                            lhsT=p_sb[:, c : c + 1],
                            rhs=et[c][:, 0, h * 512 : (h + 1) * 512],
                            start=(c == 0),
                            stop=(c == CB - 1),
                        )
                zps = sb2.tile([128, 1], FP32, tag="zps", bufs=4)
                nc.vector.tensor_reduce(
                    zps[:],
                    zp[:],
                    mybir.AxisListType.X,
                    mybir.AluOpType.add,
                )
                z_ps = uzp.tile([1, 1], FP32, tag="uz", name="z_ps")
                nc.tensor.matmul(
                    out=z_ps[0:1, 0:1],
                    lhsT=ones1[:],
                    rhs=zps[:],
                    start=True,
                    stop=True,
                )
                zr = sb2.tile([1, 1], FP32, tag="zr", bufs=4)
                nc.vector.reciprocal(out=zr[:], in_=z_ps[0:1, :])
                stg = stp.tile([1, D], FP32, tag="stg", bufs=2)
                for h in range(2):
                    nc.scalar.mul(
                        out=stg[0:1, h * 512 : (h + 1) * 512],
                        in_=u_h[h][0:1, :],
                        mul=zr[0:1, 0:1],
                    )
                nc.sync.dma_start(out=out[b : b + 1, :], in_=stg[:])

    nc.compile()
    return nc


_NC = None


def _get_nc():
    global _NC
    if _NC is None:
        _NC = build_program()
    return _NC


def make_in_maps(tokens, emb_table, W_b):
    tokens = np.asarray(tokens, dtype=np.int64)
    emb_table = np.asarray(emb_table, dtype=np.float32)
    wt_np = np.ascontiguousarray(np.asarray(W_b, dtype=np.float32).T / float(L))

    onehot_np = np.zeros((G, G * 128), dtype=np.float32)
    for j in range(G):
        onehot_np[j, j * 128 : (j + 1) * 128] = 1.0

    in_maps = []
    for m in range(N_CORES):
        tok = tokens[m * DOCS : (m + 1) * DOCS]  # [32, 512]
        # compact per-core table: only the rows this core's docs reference,
        # remapped to [0, U) so indices fit the gather ucode's int16 ids
        uniq, inv = np.unique(tok + 1, return_inverse=True)
        assert uniq.size <= U_MAX
        table_np = np.zeros((U_MAX, D), dtype=np.float32)
        table_np[: uniq.size] = emb_table[uniq]
        inv16 = inv.reshape(DOCS, L).astype(np.int16)
        # gather ucode reads idx i from partition i%16, column i//16 of a
        # [128, L/16] tile, replicated into each 16-partition group
        blk = inv16.reshape(DOCS, IW, 16).transpose(2, 0, 1)  # [16, DOCS, IW]
        idx_np = np.ascontiguousarray(
            np.tile(blk, (8, 1, 1)).reshape(128, DOCS * IW)
        )
        in_maps.append(
            {"table": table_np, "wt": wt_np, "idx": idx_np, "onehot": onehot_np}
        )
    return in_maps


def kernel(tokens, max_len, emb_table, W_b):
    assert int(max_len) == L
    nc = _get_nc()
    in_maps = make_in_maps(tokens, emb_table, W_b)
    res = run_bass_kernel_spmd(nc, in_maps, list(range(N_CORES)))
    return np.concatenate([res.results[m]["out"] for m in range(N_CORES)], axis=0)
